# revision 1
# baseline (speedup 1.0000x reference)
"""Trainium2 Bass kernel for ConvMDAformer (multi-dilation local attention).

Computation (per batch b, position n):
  qkv = x @ Wqkv.T + bqkv                        # (n, 1152)
  per dilation group g (d = g+1), head h (4 per group, hd=32):
    s_t(n) = <q(n), k(n + t*d)> * scale          # t in {-1, 0, +1}, zero-padded
    w_t(n) = exp(s_t(n)) / (6 + sum_t exp(s_t(n)))   # 6 == the six zero taps of
                                                     # the 3x3 unfold (exp(0)=1)
    out(n) = sum_t w_t(n) * v(n + t*d)
  y = out @ Wproj.T + bproj

Sharding: data-parallel, core c -> (batch c//2, sequence half c%2), halo 4/4.
Device layout: channels on partitions, sequence on the free axis, so the
tap shifts are free-dim offsets.  Head-reduction of q*k products is done with
indicator matmuls on the PE into a "spread" layout (head h of group g lands on
partition 32*h + g); softmax weights come back to all 128 channels of a group
with a stream_shuffle (mask [g]*32).

The chunk loop is software-pipelined: chunk c+1's qkv matmuls are emitted
before chunk c's attention/proj matmuls so the in-order PE stream never
starves while DVE/ACT run the softmax chain.
"""

import math
import os
from contextlib import ExitStack

import numpy as np
import ml_dtypes

# ---------------------------------------------------------------- constants
B, N, DIM = 4, 8192, 384
NUM_HEADS = 12
HEAD_DIM = DIM // NUM_HEADS          # 32
ND = 3                               # dilation groups (d = 1, 2, 3)
CD = DIM // ND                       # 128 channels per group
SCALE = HEAD_DIM ** -0.5
NCORES = 8
HALO = 4                             # left pad (>= 3; 4 keeps taps 4B aligned)
NLOC = N // 2                        # sequence positions per core
F = 1024                             # chunk width along sequence
PIECE = 512                          # psum / matmul piece width

BF16 = ml_dtypes.bfloat16

# engine split for qkv psum evacuation (o-tiles 0..8 -> ACT if in this set)
ACT_EVAC = (0, 2, 4, 6, 7, 8)


def _pieces(width, piece):
    out = []
    p0 = 0
    while p0 < width:
        out.append((p0, min(piece, width - p0)))
        p0 += piece
    return out


def build_program(n_loc=NLOC, f=F, num_devices=NCORES):
    """Builds the (single-program SPMD) Bass kernel. Returns the compiled nc."""
    from concourse import bacc
    import concourse.tile as tile
    import concourse.mybir as mybir

    dt = mybir.dt
    AF = mybir.ActivationFunctionType
    ALU = mybir.AluOpType

    assert n_loc % f == 0
    nch = n_loc // f
    fw = f + 2 * HALO
    piece = min(PIECE, f)
    qkv_pieces = _pieces(fw, piece)
    f_pieces = _pieces(f, piece)

    nc = bacc.Bacc(
        "TRN2",
        target_bir_lowering=False,
        debug=False,
        enable_asserts=False,
        num_devices=num_devices,
    )

    W = n_loc + 2 * HALO
    xt_d = nc.dram_tensor("xt", [DIM, W], dt.bfloat16, kind="ExternalInput").ap()
    wq_d = nc.dram_tensor("wqkvt", [3, 128, 3 * DIM], dt.bfloat16,
                          kind="ExternalInput").ap()
    wp_d = nc.dram_tensor("wprojt", [3, 128, DIM], dt.bfloat16,
                          kind="ExternalInput").ap()
    bias_d = nc.dram_tensor("bqkv9", [128, 9], dt.float32,
                            kind="ExternalInput").ap()
    ind_d = nc.dram_tensor("ind", [3, 128, 128], dt.bfloat16,
                           kind="ExternalInput").ap()
    y_d = nc.dram_tensor("y", [DIM, n_loc], dt.float32,
                         kind="ExternalOutput").ap()

    with TileCtx(tile, nc) as (tc, ctx):
        wpool = ctx.enter_context(tc.tile_pool(name="wpool", bufs=1))
        xpool = ctx.enter_context(tc.tile_pool(name="xpool", bufs=2))
        qkvpool = ctx.enter_context(tc.tile_pool(name="qkvpool", bufs=2))
        appool = ctx.enter_context(tc.tile_pool(name="appool", bufs=2))
        epool = ctx.enter_context(tc.tile_pool(name="epool", bufs=2))
        upool = ctx.enter_context(tc.tile_pool(name="upool", bufs=2))
        ps_qkv = ctx.enter_context(tc.tile_pool(name="ps_qkv", bufs=4, space="PSUM"))
        ps_s = ctx.enter_context(tc.tile_pool(name="ps_s", bufs=2, space="PSUM"))
        ps_o = ctx.enter_context(tc.tile_pool(name="ps_o", bufs=2, space="PSUM"))

        # ---- persistent weights
        wq = wpool.tile([128, 3, 3 * DIM], dt.bfloat16, name="wq")
        wp = wpool.tile([128, 3, DIM], dt.bfloat16, name="wp")
        for kt in range(3):
            nc.sync.dma_start(wq[:, kt, :], wq_d[kt])
            nc.sync.dma_start(wp[:, kt, :], wp_d[kt])
        bias = wpool.tile([128, 9], dt.float32, name="bias")
        nc.sync.dma_start(bias[:], bias_d[:])
        ind = wpool.tile([128, 3, 128], dt.bfloat16, name="ind")
        for g in range(3):
            nc.sync.dma_start(ind[:, g, :], ind_d[g])

        def emit_qkv(c):
            """DMA x chunk + qkv projection; returns the 9 qkv sbuf tiles."""
            xt_t = xpool.tile([128, 3, fw], dt.bfloat16, name="xt_t", tag="xt")
            for kt in range(3):
                nc.sync.dma_start(
                    xt_t[:, kt, :],
                    xt_d[kt * 128:(kt + 1) * 128, c * f: c * f + fw])
            qkv = []
            for o in range(9):
                qt = qkvpool.tile([128, fw], dt.bfloat16, name=f"qkv{o}",
                                  tag=f"qkv{o}")
                qkv.append(qt)
                # kt-outer / piece-inner: consecutive matmuls share lhsT
                pss = [ps_qkv.tile([128, piece], dt.float32, name="psq",
                                   tag="psq") for _ in qkv_pieces]
                for kt in range(3):
                    for pi, (p0, pw) in enumerate(qkv_pieces):
                        nc.tensor.matmul(
                            pss[pi][:, :pw],
                            wq[:, kt, o * 128:(o + 1) * 128],
                            xt_t[:, kt, p0:p0 + pw],
                            start=(kt == 0), stop=(kt == 2))
                for pi, (p0, pw) in enumerate(qkv_pieces):
                    if o in ACT_EVAC:
                        nc.scalar.activation(qt[:, p0:p0 + pw], pss[pi][:, :pw],
                                             AF.Identity, bias=bias[:, o:o + 1])
                    else:
                        nc.vector.tensor_scalar_add(qt[:, p0:p0 + pw],
                                                    pss[pi][:, :pw],
                                                    bias[:, o:o + 1])
            return qkv

        def emit_attn(c, qkv):
            """Attention + output projection for chunk c (qkv already done)."""
            # logits + exp for the 3 taps
            e_tiles = []
            for t in range(3):
                prods = []
                for g in range(3):
                    s = (t - 1) * (g + 1)
                    pr = appool.tile([128, f], dt.bfloat16, name=f"prod{g}",
                                     tag=f"prod{g}")
                    eng = nc.gpsimd if (g == 0 and t != 1) else nc.vector
                    eng.tensor_mul(pr[:], qkv[g][:, HALO:HALO + f],
                                   qkv[3 + g][:, HALO + s:HALO + s + f])
                    prods.append(pr)
                et = epool.tile([128, f], dt.bfloat16, name=f"e{t}", tag=f"e{t}")
                e_tiles.append(et)
                pst = [ps_s.tile([128, piece], dt.float32, name="pss",
                                 tag="pss") for _ in f_pieces]
                for g in range(3):
                    for pi, (p0, pw) in enumerate(f_pieces):
                        nc.tensor.matmul(pst[pi][:, :pw], ind[:, g, :],
                                         prods[g][:, p0:p0 + pw],
                                         start=(g == 0), stop=(g == 2))
                for pi, (p0, pw) in enumerate(f_pieces):
                    nc.scalar.activation(et[:, p0:p0 + pw], pst[pi][:, :pw],
                                         AF.Exp, scale=SCALE)

            # softmax denominator: 6 zero-taps + 3 real taps (on gpsimd)
            den = epool.tile([128, f], dt.float32, name="den", tag="den")
            nc.vector.scalar_tensor_tensor(den[:], e_tiles[0][:], 6.0,
                                           e_tiles[1][:],
                                           op0=ALU.add, op1=ALU.add)
            den2 = epool.tile([128, f], dt.float32, name="den2", tag="den2")
            nc.vector.tensor_add(den2[:], den[:], e_tiles[2][:])
            recf = epool.tile([128, f], dt.float32, name="recf", tag="recf")
            nc.vector.reciprocal_approx_fast(recf[:], den2[:])
            rec = epool.tile([128, f], dt.bfloat16, name="rec", tag="rec")
            nc.vector.tensor_copy(rec[:], recf[:])

            # normalized weights, broadcast to channels, weight V
            wts = []
            for t in range(3):
                wt = appool.tile([128, f], dt.bfloat16, name=f"wt{t}",
                                 tag=f"wt{t}")
                nc.vector.tensor_mul(wt[:], e_tiles[t][:], rec[:])
                wts.append(wt)
            u_g = []
            for g in range(3):
                uts = []
                for t in range(3):
                    bt = appool.tile([128, f], dt.bfloat16, name="bt", tag="bt")
                    nc.vector.stream_shuffle(bt[:], wts[t][:], mask=[g] * 32)
                    s = (t - 1) * (g + 1)
                    ut = appool.tile([128, f], dt.bfloat16, name=f"ut{t}",
                                     tag=f"ut{t}")
                    eng = nc.gpsimd if (g == 0 and t != 1) else nc.vector
                    eng.tensor_mul(ut[:], bt[:],
                                   qkv[6 + g][:, HALO + s:HALO + s + f])
                    uts.append(ut)
                u01 = appool.tile([128, f], dt.bfloat16, name="u01", tag="u01")
                nc.gpsimd.tensor_add(u01[:], uts[0][:], uts[1][:])
                ug = upool.tile([128, f], dt.bfloat16, name=f"u{g}", tag=f"u{g}")
                nc.vector.tensor_add(ug[:], u01[:], uts[2][:])
                u_g.append(ug)

            # output projection, evacuate psum -> sbuf -> DRAM
            for co in range(3):
                yt = upool.tile([128, f], dt.float32, name=f"y{co}",
                                tag=f"y{co}")
                pso = [ps_o.tile([128, piece], dt.float32, name="pso",
                                 tag="pso") for _ in f_pieces]
                for g in range(3):
                    for pi, (p0, pw) in enumerate(f_pieces):
                        nc.tensor.matmul(pso[pi][:, :pw],
                                         wp[:, g, co * 128:(co + 1) * 128],
                                         u_g[g][:, p0:p0 + pw],
                                         start=(g == 0), stop=(g == 2))
                for pi, (p0, pw) in enumerate(f_pieces):
                    nc.scalar.activation(yt[:, p0:p0 + pw], pso[pi][:, :pw],
                                         AF.Identity)
                nc.sync.dma_start(
                    y_d[co * 128:(co + 1) * 128, c * f: c * f + f], yt[:])

        # software pipeline: qkv(c+1) is emitted before attn(c)
        pending = None
        for c in range(nch):
            qkv = emit_qkv(c)
            if pending is not None:
                emit_attn(*pending)
            pending = (c, qkv)
        emit_attn(*pending)

    nc.compile()
    return nc


class TileCtx:
    """`with TileCtx(tile, nc) as (tc, ctx)` -> TileContext + ExitStack that
    closes (pools released) before TileContext finalizes."""

    def __init__(self, tile_mod, nc):
        self._tc_cm = tile_mod.TileContext(nc)
        self._stack = ExitStack()

    def __enter__(self):
        tc = self._tc_cm.__enter__()
        self._stack.__enter__()
        return tc, self._stack

    def __exit__(self, *exc):
        self._stack.__exit__(*exc)
        return self._tc_cm.__exit__(*exc)


# ------------------------------------------------------------ host helpers

def host_inputs(x, Wqkv, bqkv, Wproj, n_loc=NLOC):
    """Builds the per-core input dicts (and the shared weight arrays)."""
    x = np.asarray(x, dtype=np.float32)
    Wqkv = np.asarray(Wqkv, dtype=np.float32)
    bqkv = np.asarray(bqkv, dtype=np.float32)
    Wproj = np.asarray(Wproj, dtype=np.float32)

    wqkvt = np.ascontiguousarray(
        Wqkv.T.reshape(3, 128, 3 * DIM)).astype(BF16)
    wprojt = np.ascontiguousarray(
        Wproj.T.reshape(3, 128, DIM)).astype(BF16)
    bqkv9 = np.ascontiguousarray(bqkv.reshape(9, 128).T).astype(np.float32)
    ind = np.zeros((3, 128, 128), dtype=BF16)
    for g in range(3):
        for c in range(128):
            ind[g, c, 32 * (c // 32) + g] = 1

    b_all, n_all = x.shape[0], x.shape[1]
    halves = n_all // n_loc
    padded = np.zeros((b_all, n_all + 2 * HALO, x.shape[2]), dtype=np.float32)
    padded[:, HALO:HALO + n_all] = x

    in_maps = []
    for core in range(NCORES):
        b, h = divmod(core, halves)
        sl = padded[b, h * n_loc: h * n_loc + n_loc + 2 * HALO]
        xt = np.ascontiguousarray(sl.T).astype(BF16)
        in_maps.append({
            "xt": xt,
            "wqkvt": wqkvt,
            "wprojt": wprojt,
            "bqkv9": bqkv9,
            "ind": ind,
        })
    return in_maps


def assemble_output(results, bproj, n_loc=NLOC):
    bproj = np.asarray(bproj, dtype=np.float32)
    out = np.empty((B, N, DIM), dtype=np.float32)
    halves = N // n_loc
    for core in range(NCORES):
        b, h = divmod(core, halves)
        out[b, h * n_loc:(h + 1) * n_loc, :] = results[core]["y"].T
    out += bproj
    return out


def kernel(x, Wqkv, bqkv, Wproj, bproj):
    from concourse import bass_utils

    nc = build_program()
    in_maps = host_inputs(x, Wqkv, bqkv, Wproj)
    trace = bool(int(os.environ.get("KERNEL_TRACE", "0")))
    res = bass_utils.run_bass_kernel_spmd(
        nc, in_maps, core_ids=list(range(NCORES)), trace=trace)
    kernel.last_result = res
    return assemble_output(res.results, bproj)



# revision 4
# speedup vs baseline: 1.1165x; 1.1165x over previous
"""Trainium2 Bass kernel for ConvMDAformer (multi-dilation local attention).

Computation (per batch b, position n):
  qkv = x @ Wqkv.T + bqkv                        # (n, 1152)
  per dilation group g (d = g+1), head h (4 per group, hd=32):
    s_t(n) = <q(n), k(n + t*d)> * scale          # t in {-1, 0, +1}, zero-padded
    w_t(n) = exp(s_t(n)) / (6 + sum_t exp(s_t(n)))   # 6 == the six zero taps of
                                                     # the 3x3 unfold (exp(0)=1)
    out(n) = sum_t w_t(n) * v(n + t*d)
  y = out @ Wproj.T + bproj

Sharding: data-parallel, core c -> (batch c//2, sequence half c%2), halo 4/4.
Channels on partitions, sequence on the free axis; tap shifts are free-dim
offsets.  Head-reduction of q*k products is done with indicator matmuls on the
PE into a "spread" layout (head h of group g lands on partition 32*h + g);
softmax weights come back to the 128 channels of a group via stream_shuffle.

Schedule (iter c): logits+exp(c) | qkv(c+1) | softmax chain(c) | prods(c+1) |
proj(c).  Multi-tap elementwise ops are merged into single DVE instructions
via strided/broadcast access patterns; PSUM evacuations are single 1024-wide
ACT ops over 2-bank tiles; gpsimd takes group-0 off-center taps (it cannot
touch PSUM).
"""

import math
import os
from contextlib import ExitStack

import numpy as np
import ml_dtypes

# ---------------------------------------------------------------- constants
B, N, DIM = 4, 8192, 384
NUM_HEADS = 12
HEAD_DIM = DIM // NUM_HEADS          # 32
ND = 3                               # dilation groups (d = 1, 2, 3)
CD = DIM // ND                       # 128 channels per group
SCALE = HEAD_DIM ** -0.5
NCORES = 8
HALO = 4                             # halo columns on each side
NLOC = N // 2                        # sequence positions per core
F = 1024                             # chunk width along sequence
FW = F + 2 * HALO                    # 1032
PIECE = 512

BF16 = ml_dtypes.bfloat16

F_PIECES = ((0, 512), (512, 512))


def build_program(n_loc=NLOC, num_devices=NCORES):
    from concourse import bacc
    import concourse.tile as tile
    import concourse.mybir as mybir
    from concourse.ap import AP

    dt = mybir.dt
    AF = mybir.ActivationFunctionType
    ALU = mybir.AluOpType

    assert n_loc % F == 0
    nch = n_loc // F

    nc = bacc.Bacc(
        "TRN2",
        target_bir_lowering=False,
        debug=False,
        enable_asserts=False,
        num_devices=num_devices,
    )

    W = n_loc + 2 * HALO
    xt_d = nc.dram_tensor("xt", [DIM, W], dt.bfloat16, kind="ExternalInput").ap()
    wq_d = nc.dram_tensor("wqkvt", [3, 128, 3 * DIM], dt.bfloat16,
                          kind="ExternalInput").ap()
    wp_d = nc.dram_tensor("wprojt", [3, 128, DIM], dt.bfloat16,
                          kind="ExternalInput").ap()
    bias_d = nc.dram_tensor("bqkv9", [128, 9], dt.float32,
                            kind="ExternalInput").ap()
    ind_d = nc.dram_tensor("ind", [3, 128, 128], dt.bfloat16,
                           kind="ExternalInput").ap()
    y_d = nc.dram_tensor("y", [DIM, n_loc], dt.float32,
                         kind="ExternalOutput").ap()

    def tap_ap(base2d, d, ntap=3, width=F):
        """[128, ntap, width] view of a [128, fw] slice: tap t starts at
        HALO - d + t*d (shift (t-1)*d relative to the chunk interior)."""
        return AP(base2d.tensor, base2d.offset + (HALO - d),
                  [list(base2d.ap[0]), [d, ntap], [1, width]])

    def bcast3(base2d, width=F):
        return base2d.unsqueeze(1).broadcast_to([128, 3, width])

    with TileCtx(tile, nc) as (tc, ctx):
        wpool = ctx.enter_context(tc.tile_pool(name="wpool", bufs=1))
        xpool = ctx.enter_context(tc.tile_pool(name="xpool", bufs=2))
        qpool = ctx.enter_context(tc.tile_pool(name="qpool", bufs=1))
        kvpool = ctx.enter_context(tc.tile_pool(name="kvpool", bufs=2))
        prpool = ctx.enter_context(tc.tile_pool(name="prpool", bufs=2))
        epool = ctx.enter_context(tc.tile_pool(name="epool", bufs=1))
        mpool = ctx.enter_context(tc.tile_pool(name="mpool", bufs=1))
        ytpool = ctx.enter_context(tc.tile_pool(name="ytpool", bufs=2))
        ps_qkv = ctx.enter_context(tc.tile_pool(name="ps_qkv", bufs=2, space="PSUM"))
        ps_s = ctx.enter_context(tc.tile_pool(name="ps_s", bufs=2, space="PSUM"))
        ps_o = ctx.enter_context(tc.tile_pool(name="ps_o", bufs=2, space="PSUM"))

        # ---- persistent weights
        wq = wpool.tile([128, 3, 3 * DIM], dt.bfloat16, name="wq")
        wp = wpool.tile([128, 3, DIM], dt.bfloat16, name="wp")
        for kt in range(3):
            nc.sync.dma_start(wq[:, kt, :], wq_d[kt])
            nc.sync.dma_start(wp[:, kt, :], wp_d[kt])
        bias = wpool.tile([128, 9], dt.float32, name="bias")
        nc.sync.dma_start(bias[:], bias_d[:])
        ind = wpool.tile([128, 3, 128], dt.bfloat16, name="ind")
        for g in range(3):
            nc.sync.dma_start(ind[:, g, :], ind_d[g])

        def emit_x_dma(c):
            xt_t = xpool.tile([128, 3, FW], dt.bfloat16, name="xt_t", tag="xt")
            for kt in range(3):
                nc.sync.dma_start(
                    xt_t[:, kt, :],
                    xt_d[kt * 128:(kt + 1) * 128, c * F: c * F + FW])
            return xt_t

        def emit_qkv(c, xt_t):
            """qkv projection for chunk c; q at F cols, k/v at FW cols
            (k/v halo tails via one small psum tile)."""
            q = qpool.tile([128, 3, F], dt.bfloat16, name="q", tag="q")
            kv = kvpool.tile([128, 6, FW], dt.bfloat16, name="kv", tag="kv")
            for o in range(3):          # q tiles
                ps = ps_qkv.tile([128, 1024], dt.float32, name="psq", tag="psq")
                for (p0, pw) in F_PIECES:
                    for kt in range(3):
                        nc.tensor.matmul(
                            ps[:, p0:p0 + pw],
                            wq[:, kt, o * 128:(o + 1) * 128],
                            xt_t[:, kt, HALO + p0: HALO + p0 + pw],
                            start=(kt == 0), stop=(kt == 2))
                nc.scalar.activation(q[:, o, :], ps[:], AF.Identity,
                                     bias=bias[:, o:o + 1])
            tails = ps_s.tile([128, 512], dt.float32, name="tails", tag="pss")
            for j in range(6):          # k tiles (j 0..2), v tiles (j 3..5)
                o = 3 + j
                ps = ps_qkv.tile([128, 1024], dt.float32, name="psq", tag="psq")
                for (p0, pw) in F_PIECES:
                    for kt in range(3):
                        nc.tensor.matmul(
                            ps[:, p0:p0 + pw],
                            wq[:, kt, o * 128:(o + 1) * 128],
                            xt_t[:, kt, p0: p0 + pw],
                            start=(kt == 0), stop=(kt == 2))
                nc.scalar.activation(kv[:, j, 0:1024], ps[:], AF.Identity,
                                     bias=bias[:, o:o + 1])
                for kt in range(3):     # 8-col halo tail
                    nc.tensor.matmul(
                        tails[:, 8 * j: 8 * j + 8],
                        wq[:, kt, o * 128:(o + 1) * 128],
                        xt_t[:, kt, 1024:1032],
                        start=(kt == 0), stop=(kt == 2))
                if j == 2 or j == 5:    # evac k tails early for prods(c+1)
                    lo = 0 if j == 2 else 3
                    src = AP(tails[:].tensor, tails[:].offset + 8 * lo,
                             [list(tails[:].ap[0]), [8, 3], [1, 8]])
                    nc.scalar.activation(kv[:, lo:lo + 3, 1024:1032], src,
                                         AF.Identity)
            return q, kv

        def emit_prods(c, q, kv):
            """q (x) k-shifted products, all 9 (g, t); bias=0 so halo cols of
            k are exact zeros at sequence edges."""
            prods = prpool.tile([128, 3, 3, F], dt.bfloat16, name="prods",
                                tag="prods")
            nc.vector.tensor_mul(prods[:, 0, 1, :], q[:, 0, :],
                                 kv[:, 0, HALO:HALO + F])
            for g in (1, 2):
                nc.vector.tensor_mul(prods[:, g, :, :], bcast3(q[:, g, :]),
                                     tap_ap(kv[:, g, :], g + 1))
            nc.gpsimd.tensor_mul(prods[:, 0, 0, :], q[:, 0, :],
                                 kv[:, 0, HALO - 1:HALO - 1 + F])
            nc.gpsimd.tensor_mul(prods[:, 0, 2, :], q[:, 0, :],
                                 kv[:, 0, HALO + 1:HALO + 1 + F])
            return prods

        def emit_logits(c, prods):
            """Indicator matmuls (head-reduce into spread layout) + exp."""
            e = epool.tile([128, 3, F], dt.bfloat16, name="e", tag="e")
            for t in range(3):
                for (p0, pw) in F_PIECES:
                    st = ps_s.tile([128, 512], dt.float32, name="pss",
                                   tag="pss")
                    for g in range(3):
                        nc.tensor.matmul(st[:, :pw], ind[:, g, :],
                                         prods[:, g, t, p0:p0 + pw],
                                         start=(g == 0), stop=(g == 2))
                    nc.scalar.activation(e[:, t, p0:p0 + pw], st[:, :pw],
                                         AF.Exp, scale=SCALE)
            return e

        def emit_den(c, e):
            e01 = epool.tile([128, F], dt.bfloat16, name="e01", tag="e01")
            nc.vector.tensor_add(e01[:], e[:, 0, :], e[:, 1, :])
            den = epool.tile([128, F], dt.float32, name="den", tag="den")
            nc.vector.scalar_tensor_tensor(den[:], e01[:], 6.0, e[:, 2, :],
                                           op0=ALU.add, op1=ALU.add)
            recf = epool.tile([128, F], dt.float32, name="recf", tag="recf")
            nc.vector.reciprocal_approx_fast(recf[:], den[:])
            rec = epool.tile([128, F], dt.bfloat16, name="rec", tag="rec")
            nc.scalar.copy(rec[:], recf[:])
            return rec

        def emit_wv(c, e, rec, kv):
            """Normalized weights, broadcast to channels, weight V, tap-sum."""
            wts = mpool.tile([128, 3, F], dt.bfloat16, name="wts", tag="wts")
            nc.vector.tensor_mul(wts[:], e[:], bcast3(rec[:]))
            bt = mpool.tile([128, 3, 3, F], dt.bfloat16, name="bt", tag="bt")
            for g in range(3):
                for t in range(3):
                    nc.vector.stream_shuffle(bt[:, g, t, :], wts[:, t, :],
                                             mask=[g] * 32)
            ut = mpool.tile([128, 3, 3, F], dt.bfloat16, name="ut", tag="ut")
            nc.gpsimd.tensor_mul(ut[:, 0, 0, :], bt[:, 0, 0, :],
                                 kv[:, 3, HALO - 1:HALO - 1 + F])
            nc.vector.tensor_mul(ut[:, 0, 1, :], bt[:, 0, 1, :],
                                 kv[:, 3, HALO:HALO + F])
            for g in (1, 2):
                nc.vector.tensor_mul(ut[:, g, :, :], bt[:, g, :, :],
                                     tap_ap(kv[:, 3 + g, :], g + 1))
            nc.gpsimd.tensor_mul(ut[:, 0, 2, :], bt[:, 0, 2, :],
                                 kv[:, 3, HALO + 1:HALO + 1 + F])
            u01 = mpool.tile([128, 3, F], dt.bfloat16, name="u01", tag="u01")
            u = mpool.tile([128, 3, F], dt.bfloat16, name="u", tag="u")
            nc.gpsimd.tensor_add(u01[:, 0, :], ut[:, 0, 0, :], ut[:, 0, 1, :])
            nc.vector.tensor_add(u01[:, 1:3, :], ut[:, 1:3, 0, :],
                                 ut[:, 1:3, 1, :])
            nc.vector.tensor_add(u[:, 1:3, :], u01[:, 1:3, :],
                                 ut[:, 1:3, 2, :])
            nc.vector.tensor_add(u[:, 0, :], u01[:, 0, :], ut[:, 0, 2, :])
            return u

        def emit_proj(c, u):
            for co in range(3):
                yt = ytpool.tile([128, F], dt.float32, name=f"y{co}",
                                 tag=f"y{co}")
                for (p0, pw) in F_PIECES:
                    po = ps_o.tile([128, 512], dt.float32, name="pso",
                                   tag="pso")
                    for gi, g in enumerate((1, 2, 0)):
                        nc.tensor.matmul(po[:, :pw],
                                         wp[:, g, co * 128:(co + 1) * 128],
                                         u[:, g, p0:p0 + pw],
                                         start=(gi == 0), stop=(gi == 2))
                    nc.scalar.activation(yt[:, p0:p0 + pw], po[:, :pw],
                                         AF.Identity)
                nc.sync.dma_start(
                    y_d[co * 128:(co + 1) * 128, c * F: c * F + F], yt[:])

        # -------- software pipeline --------
        xs = {0: emit_x_dma(0)}
        if nch > 1:
            xs[1] = emit_x_dma(1)
        q, kv = emit_qkv(0, xs[0])
        prods = emit_prods(0, q, kv)
        state = (q, kv, prods)

        for c in range(nch):
            q, kv, prods = state
            if c + 2 < nch:
                xs[c + 2] = emit_x_dma(c + 2)
            e = emit_logits(c, prods)
            rec = emit_den(c, e)
            if c + 1 < nch:
                qn, kvn = emit_qkv(c + 1, xs[c + 1])
            u = emit_wv(c, e, rec, kv)
            if c + 1 < nch:
                prodsn = emit_prods(c + 1, qn, kvn)
                state = (qn, kvn, prodsn)
            emit_proj(c, u)

    nc.compile()
    return nc


class TileCtx:
    """`with TileCtx(tile, nc) as (tc, ctx)` -> TileContext + ExitStack that
    closes (pools released) before TileContext finalizes."""

    def __init__(self, tile_mod, nc):
        self._tc_cm = tile_mod.TileContext(nc)
        self._stack = ExitStack()

    def __enter__(self):
        tc = self._tc_cm.__enter__()
        self._stack.__enter__()
        return tc, self._stack

    def __exit__(self, *exc):
        self._stack.__exit__(*exc)
        return self._tc_cm.__exit__(*exc)


# ------------------------------------------------------------ host helpers

def host_inputs(x, Wqkv, bqkv, Wproj, n_loc=NLOC):
    """Builds the per-core input dicts (and the shared weight arrays)."""
    x = np.asarray(x, dtype=np.float32)
    Wqkv = np.asarray(Wqkv, dtype=np.float32)
    bqkv = np.asarray(bqkv, dtype=np.float32)
    Wproj = np.asarray(Wproj, dtype=np.float32)

    wqkvt = np.ascontiguousarray(
        Wqkv.T.reshape(3, 128, 3 * DIM)).astype(BF16)
    wprojt = np.ascontiguousarray(
        Wproj.T.reshape(3, 128, DIM)).astype(BF16)
    bqkv9 = np.ascontiguousarray(bqkv.reshape(9, 128).T).astype(np.float32)
    ind = np.zeros((3, 128, 128), dtype=BF16)
    for g in range(3):
        for c in range(128):
            ind[g, c, 32 * (c // 32) + g] = 1

    b_all, n_all = x.shape[0], x.shape[1]
    halves = n_all // n_loc
    padded = np.zeros((b_all, n_all + 2 * HALO, x.shape[2]), dtype=np.float32)
    padded[:, HALO:HALO + n_all] = x

    in_maps = []
    for core in range(NCORES):
        b, h = divmod(core, halves)
        sl = padded[b, h * n_loc: h * n_loc + n_loc + 2 * HALO]
        xt = np.ascontiguousarray(sl.T).astype(BF16)
        in_maps.append({
            "xt": xt,
            "wqkvt": wqkvt,
            "wprojt": wprojt,
            "bqkv9": bqkv9,
            "ind": ind,
        })
    return in_maps


def assemble_output(results, bproj, n_loc=NLOC):
    bproj = np.asarray(bproj, dtype=np.float32)
    out = np.empty((B, N, DIM), dtype=np.float32)
    halves = N // n_loc
    for core in range(NCORES):
        b, h = divmod(core, halves)
        out[b, h * n_loc:(h + 1) * n_loc, :] = results[core]["y"].T
    out += bproj
    return out


def kernel(x, Wqkv, bqkv, Wproj, bproj):
    from concourse import bass_utils

    nc = build_program()
    in_maps = host_inputs(x, Wqkv, bqkv, Wproj)
    trace = bool(int(os.environ.get("KERNEL_TRACE", "0")))
    res = bass_utils.run_bass_kernel_spmd(
        nc, in_maps, core_ids=list(range(NCORES)), trace=trace)
    kernel.last_result = res
    return assemble_output(res.results, bproj)


# revision 8
# speedup vs baseline: 1.1774x; 1.0545x over previous
"""Trainium2 Bass kernel for ConvMDAformer (multi-dilation local attention).

Computation (per batch b, position n):
  qkv = x @ Wqkv.T + bqkv                        # (n, 1152)
  per dilation group g (d = g+1), head h (4 per group, hd=32):
    s_t(n) = <q(n), k(n + t*d)> * scale          # t in {-1, 0, +1}, zero-padded
    w_t(n) = exp(s_t(n)) / (6 + sum_t exp(s_t(n)))   # 6 == the six zero taps of
                                                     # the 3x3 unfold (exp(0)=1)
    out(n) = sum_t w_t(n) * v(n + t*d)
  y = out @ Wproj.T + bproj

Sharding: data-parallel, core c -> (batch c//2, sequence half c%2), halo 4/4.
Channels on partitions, sequence on the free axis; tap shifts are free-dim
offsets.  Head-reduction of q*k products is done with indicator matmuls on the
PE into a "spread" layout (head h of group g lands on partition 32*h + g);
softmax weights come back to the 128 channels of a group via stream_shuffle.

Schedule (iter c): logits+exp(c) | qkv(c+1) | softmax chain(c) | prods(c+1) |
proj(c).  Multi-tap elementwise ops are merged into single DVE instructions
via strided/broadcast access patterns; PSUM evacuations are single 1024-wide
ACT ops over 2-bank tiles; gpsimd takes group-0 off-center taps (it cannot
touch PSUM).
"""

import math
import os
from contextlib import ExitStack

import numpy as np
import ml_dtypes

# ---------------------------------------------------------------- constants
B, N, DIM = 4, 8192, 384
NUM_HEADS = 12
HEAD_DIM = DIM // NUM_HEADS          # 32
ND = 3                               # dilation groups (d = 1, 2, 3)
CD = DIM // ND                       # 128 channels per group
SCALE = HEAD_DIM ** -0.5
NCORES = 8
HALO = 4                             # halo columns on each side
NLOC = N // 2                        # sequence positions per core
F = 1024                             # chunk width along sequence
FW = F + 2 * HALO                    # 1032
PIECE = 512

BF16 = ml_dtypes.bfloat16

F_PIECES = ((0, 512), (512, 512))


def build_program(n_loc=NLOC, num_devices=NCORES):
    from concourse import bacc
    import concourse.tile as tile
    import concourse.mybir as mybir
    from concourse.ap import AP

    dt = mybir.dt
    AF = mybir.ActivationFunctionType
    ALU = mybir.AluOpType

    assert n_loc % F == 0
    nch = n_loc // F

    nc = bacc.Bacc(
        "TRN2",
        target_bir_lowering=False,
        debug=False,
        enable_asserts=False,
        num_devices=num_devices,
    )

    W = n_loc + 2 * HALO
    xt_d = nc.dram_tensor("xt", [DIM, W], dt.bfloat16, kind="ExternalInput").ap()
    wq_d = nc.dram_tensor("wqkvt", [3, 128, 3 * DIM], dt.bfloat16,
                          kind="ExternalInput").ap()
    wp_d = nc.dram_tensor("wprojt", [3, 128, DIM], dt.bfloat16,
                          kind="ExternalInput").ap()
    bias_d = nc.dram_tensor("bqkv9", [128, 9], dt.float32,
                            kind="ExternalInput").ap()
    ind_d = nc.dram_tensor("ind", [3, 128, 128], dt.bfloat16,
                           kind="ExternalInput").ap()
    y_d = nc.dram_tensor("y", [DIM, n_loc], dt.float32,
                         kind="ExternalOutput").ap()

    def tap_ap(base2d, d, ntap=3, width=F):
        """[128, ntap, width] view of a [128, fw] slice: tap t starts at
        HALO - d + t*d (shift (t-1)*d relative to the chunk interior)."""
        return AP(base2d.tensor, base2d.offset + (HALO - d),
                  [list(base2d.ap[0]), [d, ntap], [1, width]])

    def bcast3(base2d, width=F):
        return base2d.unsqueeze(1).broadcast_to([128, 3, width])

    with TileCtx(tile, nc) as (tc, ctx):
        wpool = ctx.enter_context(tc.tile_pool(name="wpool", bufs=1))
        xpool = ctx.enter_context(tc.tile_pool(name="xpool", bufs=2))
        qpool = ctx.enter_context(tc.tile_pool(name="qpool", bufs=1))
        kvpool = ctx.enter_context(tc.tile_pool(name="kvpool", bufs=3))
        prpool = ctx.enter_context(tc.tile_pool(name="prpool", bufs=2))
        epool = ctx.enter_context(tc.tile_pool(name="epool", bufs=2))
        dpool = ctx.enter_context(tc.tile_pool(name="dpool", bufs=1))
        mpool = ctx.enter_context(tc.tile_pool(name="mpool", bufs=1))
        ytpool = ctx.enter_context(tc.tile_pool(name="ytpool", bufs=2))
        ps_qkv = ctx.enter_context(tc.tile_pool(name="ps_qkv", bufs=2, space="PSUM"))
        ps_s = ctx.enter_context(tc.tile_pool(name="ps_s", bufs=2, space="PSUM"))
        ps_o = ctx.enter_context(tc.tile_pool(name="ps_o", bufs=2, space="PSUM"))

        # ---- persistent weights
        wq = wpool.tile([128, 3, 3 * DIM], dt.bfloat16, name="wq")
        wp = wpool.tile([128, 3, DIM], dt.bfloat16, name="wp")
        for kt in range(3):
            nc.sync.dma_start(wq[:, kt, :], wq_d[kt])
            nc.sync.dma_start(wp[:, kt, :], wp_d[kt])
        bias = wpool.tile([128, 9], dt.float32, name="bias")
        nc.sync.dma_start(bias[:], bias_d[:])
        ind = wpool.tile([128, 3, 128], dt.bfloat16, name="ind")
        for g in range(3):
            nc.sync.dma_start(ind[:, g, :], ind_d[g])

        def emit_x_dma(c):
            xt_t = xpool.tile([128, 3, FW], dt.bfloat16, name="xt_t", tag="xt")
            for kt in range(3):
                nc.sync.dma_start(
                    xt_t[:, kt, :],
                    xt_d[kt * 128:(kt + 1) * 128, c * F: c * F + FW])
            return xt_t

        def emit_qkv(c, xt_t):
            """qkv projection for chunk c; q at F cols, k/v at FW cols
            (k/v halo tails via one small psum tile)."""
            q = qpool.tile([128, 3, F], dt.bfloat16, name="q", tag="q")
            kv = kvpool.tile([128, 6, FW], dt.bfloat16, name="kv", tag="kv")
            for o in range(3):          # q tiles
                ps = ps_qkv.tile([128, 1024], dt.float32, name="psq", tag="psq")
                for (p0, pw) in F_PIECES:
                    for kt in range(3):
                        nc.tensor.matmul(
                            ps[:, p0:p0 + pw],
                            wq[:, kt, o * 128:(o + 1) * 128],
                            xt_t[:, kt, HALO + p0: HALO + p0 + pw],
                            start=(kt == 0), stop=(kt == 2))
                nc.scalar.activation(q[:, o, :], ps[:], AF.Identity,
                                     bias=bias[:, o:o + 1])
            tails = ps_s.tile([128, 512], dt.float32, name="tails", tag="pss")
            for j in range(6):          # k tiles (j 0..2), v tiles (j 3..5)
                o = 3 + j
                ps = ps_qkv.tile([128, 1024], dt.float32, name="psq", tag="psq")
                for (p0, pw) in F_PIECES:
                    for kt in range(3):
                        nc.tensor.matmul(
                            ps[:, p0:p0 + pw],
                            wq[:, kt, o * 128:(o + 1) * 128],
                            xt_t[:, kt, p0: p0 + pw],
                            start=(kt == 0), stop=(kt == 2))
                nc.scalar.activation(kv[:, j, 0:1024], ps[:], AF.Identity,
                                     bias=bias[:, o:o + 1])
                for kt in range(3):     # 8-col halo tail
                    nc.tensor.matmul(
                        tails[:, 8 * j: 8 * j + 8],
                        wq[:, kt, o * 128:(o + 1) * 128],
                        xt_t[:, kt, 1024:1032],
                        start=(kt == 0), stop=(kt == 2))
                if j == 2 or j == 5:    # evac k tails early for prods(c+1)
                    lo = 0 if j == 2 else 3
                    src = AP(tails[:].tensor, tails[:].offset + 8 * lo,
                             [list(tails[:].ap[0]), [8, 3], [1, 8]])
                    nc.scalar.activation(kv[:, lo:lo + 3, 1024:1032], src,
                                         AF.Identity)
            return q, kv

        def emit_prods(c, q, kv):
            """q (x) k-shifted products, all 9 (g, t); bias=0 so halo cols of
            k are exact zeros at sequence edges.  Odd-offset taps (dilations
            1 and 3, t != 1) break DVE 2x alignment -> gpsimd."""
            prods = prpool.tile([128, 3, 3, F], dt.bfloat16, name="prods",
                                tag="prods")
            nc.vector.tensor_mul(prods[:, 0, 1, :], q[:, 0, :],
                                 kv[:, 0, HALO:HALO + F])
            nc.vector.tensor_mul(prods[:, 1, :, :], bcast3(q[:, 1, :]),
                                 tap_ap(kv[:, 1, :], 2))
            nc.vector.tensor_mul(prods[:, 2, 1, :], q[:, 2, :],
                                 kv[:, 2, HALO:HALO + F])
            for g, t in ((0, 0), (0, 2), (2, 0), (2, 2)):
                d = g + 1
                nc.gpsimd.tensor_mul(
                    prods[:, g, t, :], q[:, g, :],
                    kv[:, g, HALO + (t - 1) * d:HALO + (t - 1) * d + F])
            return prods

        def emit_logits(c, prods):
            """Indicator matmuls (head-reduce into spread layout) + exp."""
            e = epool.tile([128, 3, F], dt.bfloat16, name="e", tag="e")
            for t in range(3):
                for (p0, pw) in F_PIECES:
                    st = ps_s.tile([128, 512], dt.float32, name="pss",
                                   tag="pss")
                    for g in range(3):
                        nc.tensor.matmul(st[:, :pw], ind[:, g, :],
                                         prods[:, g, t, p0:p0 + pw],
                                         start=(g == 0), stop=(g == 2))
                    nc.scalar.activation(e[:, t, p0:p0 + pw], st[:, :pw],
                                         AF.Exp, scale=SCALE)
            return e

        def emit_den_dve(c, e):
            e01 = dpool.tile([128, F], dt.bfloat16, name="e01", tag="e01")
            nc.vector.tensor_add(e01[:], e[:, 0, :], e[:, 1, :])
            den = dpool.tile([128, F], dt.float32, name="den", tag="den")
            nc.vector.scalar_tensor_tensor(den[:], e01[:], 6.0, e[:, 2, :],
                                           op0=ALU.add, op1=ALU.add)
            recf = dpool.tile([128, F], dt.float32, name="recf", tag="recf")
            nc.vector.reciprocal_approx_fast(recf[:], den[:])
            return recf

        def emit_rec_act(c, recf):
            rec = epool.tile([128, F], dt.bfloat16, name="rec", tag="rec")
            nc.scalar.copy(rec[:], recf[:])
            return rec

        def emit_wv(c, e, rec, kv):
            """Normalized weights, broadcast to channels, weight V, tap-sum.
            Runs one iteration after its chunk's logits (all inputs ready)."""
            wts = mpool.tile([128, 3, F], dt.bfloat16, name="wts", tag="wts")
            nc.vector.tensor_mul(wts[:], e[:], bcast3(rec[:]))
            bt = mpool.tile([128, 3, 3, F], dt.bfloat16, name="bt", tag="bt")
            for g in range(3):
                nc.vector.stream_shuffle(bt[:, g, :, :], wts[:], mask=[g] * 32)
            ut = mpool.tile([128, 3, 3, F], dt.bfloat16, name="ut", tag="ut")
            for g, t in ((0, 0), (0, 2), (2, 0), (2, 2)):
                d = g + 1
                nc.gpsimd.tensor_mul(
                    ut[:, g, t, :], bt[:, g, t, :],
                    kv[:, 3 + g, HALO + (t - 1) * d:HALO + (t - 1) * d + F])
            nc.vector.tensor_mul(ut[:, 0, 1, :], bt[:, 0, 1, :],
                                 kv[:, 3, HALO:HALO + F])
            nc.vector.tensor_mul(ut[:, 1, :, :], bt[:, 1, :, :],
                                 tap_ap(kv[:, 4, :], 2))
            nc.vector.tensor_mul(ut[:, 2, 1, :], bt[:, 2, 1, :],
                                 kv[:, 5, HALO:HALO + F])
            u01 = mpool.tile([128, 3, F], dt.bfloat16, name="u01", tag="u01")
            u = mpool.tile([128, 3, F], dt.bfloat16, name="u", tag="u")
            nc.vector.tensor_add(u01[:, :, :], ut[:, :, 0, :], ut[:, :, 1, :])
            nc.vector.tensor_add(u[:, :, :], u01[:, :, :], ut[:, :, 2, :])
            return u

        def emit_proj(c, u):
            for co in range(3):
                yt = ytpool.tile([128, F], dt.float32, name=f"y{co}",
                                 tag=f"y{co}")
                for (p0, pw) in F_PIECES:
                    po = ps_o.tile([128, 512], dt.float32, name="pso",
                                   tag="pso")
                    for gi, g in enumerate((1, 2, 0)):
                        nc.tensor.matmul(po[:, :pw],
                                         wp[:, g, co * 128:(co + 1) * 128],
                                         u[:, g, p0:p0 + pw],
                                         start=(gi == 0), stop=(gi == 2))
                    nc.scalar.activation(yt[:, p0:p0 + pw], po[:, :pw],
                                         AF.Identity)
                nc.sync.dma_start(
                    y_d[co * 128:(co + 1) * 128, c * F: c * F + F], yt[:])

        # -------- software pipeline --------
        # iter c runs: logits/exp(c) | wv+proj(c-1) | qkv(c+1) | den(c) |
        # prods(c+1).  wv/proj are delayed one chunk so every engine enters
        # the iteration with its inputs already computed.
        xs = {0: emit_x_dma(0)}
        if nch > 1:
            xs[1] = emit_x_dma(1)
        q, kv = emit_qkv(0, xs[0])
        prods = emit_prods(0, q, kv)
        front = None                  # (c, e, rec, kv) awaiting wv+proj

        for c in range(nch):
            if c + 2 < nch:
                xs[c + 2] = emit_x_dma(c + 2)
            e = emit_logits(c, prods)
            if front is not None:
                u = emit_wv(front[0], front[1], front[2], front[3])
            recf = emit_den_dve(c, e)
            if c + 1 < nch:
                qn, kvn = emit_qkv(c + 1, xs[c + 1])
            if front is not None:
                emit_proj(front[0], u)
            rec = emit_rec_act(c, recf)
            if c + 1 < nch:
                prodsn = emit_prods(c + 1, qn, kvn)
            front = (c, e, rec, kv)
            if c + 1 < nch:
                q, kv, prods = qn, kvn, prodsn

        u = emit_wv(front[0], front[1], front[2], front[3])
        emit_proj(front[0], u)

    nc.compile()
    return nc


class TileCtx:
    """`with TileCtx(tile, nc) as (tc, ctx)` -> TileContext + ExitStack that
    closes (pools released) before TileContext finalizes."""

    def __init__(self, tile_mod, nc):
        self._tc_cm = tile_mod.TileContext(nc)
        self._stack = ExitStack()

    def __enter__(self):
        tc = self._tc_cm.__enter__()
        self._stack.__enter__()
        return tc, self._stack

    def __exit__(self, *exc):
        self._stack.__exit__(*exc)
        return self._tc_cm.__exit__(*exc)


# ------------------------------------------------------------ host helpers

def host_inputs(x, Wqkv, bqkv, Wproj, n_loc=NLOC):
    """Builds the per-core input dicts (and the shared weight arrays)."""
    x = np.asarray(x, dtype=np.float32)
    Wqkv = np.asarray(Wqkv, dtype=np.float32)
    bqkv = np.asarray(bqkv, dtype=np.float32)
    Wproj = np.asarray(Wproj, dtype=np.float32)

    wqkvt = np.ascontiguousarray(
        Wqkv.T.reshape(3, 128, 3 * DIM)).astype(BF16)
    wprojt = np.ascontiguousarray(
        Wproj.T.reshape(3, 128, DIM)).astype(BF16)
    bqkv9 = np.ascontiguousarray(bqkv.reshape(9, 128).T).astype(np.float32)
    ind = np.zeros((3, 128, 128), dtype=BF16)
    for g in range(3):
        for c in range(128):
            ind[g, c, 32 * (c // 32) + g] = 1

    b_all, n_all = x.shape[0], x.shape[1]
    halves = n_all // n_loc
    padded = np.zeros((b_all, n_all + 2 * HALO, x.shape[2]), dtype=np.float32)
    padded[:, HALO:HALO + n_all] = x

    in_maps = []
    for core in range(NCORES):
        b, h = divmod(core, halves)
        sl = padded[b, h * n_loc: h * n_loc + n_loc + 2 * HALO]
        xt = np.ascontiguousarray(sl.T).astype(BF16)
        in_maps.append({
            "xt": xt,
            "wqkvt": wqkvt,
            "wprojt": wprojt,
            "bqkv9": bqkv9,
            "ind": ind,
        })
    return in_maps


def assemble_output(results, bproj, n_loc=NLOC):
    bproj = np.asarray(bproj, dtype=np.float32)
    out = np.empty((B, N, DIM), dtype=np.float32)
    halves = N // n_loc
    for core in range(NCORES):
        b, h = divmod(core, halves)
        out[b, h * n_loc:(h + 1) * n_loc, :] = results[core]["y"].T
    out += bproj
    return out


def kernel(x, Wqkv, bqkv, Wproj, bproj):
    from concourse import bass_utils

    nc = build_program()
    in_maps = host_inputs(x, Wqkv, bqkv, Wproj)
    trace = bool(int(os.environ.get("KERNEL_TRACE", "0")))
    res = bass_utils.run_bass_kernel_spmd(
        nc, in_maps, core_ids=list(range(NCORES)), trace=trace)
    kernel.last_result = res
    return assemble_output(res.results, bproj)
